# revision 1
# baseline (speedup 1.0000x reference)
"""AttentionBlock kernel for Trainium2, 8-way batch-parallel.

Per core (one image, x [C=128, N=16384] fp32) the whole block collapses to an
image-dependent affine map:

    out = (I + W_out @ W_sm @ W_in @ diag(a)) @ x + b_fin

where a/b come from the GroupNorm stats and W_sm is the per-head softmax of
scores derived from the Gram matrix Gx = x @ x.T (spatial axis contracted).
All data-dependent nonlinearity (stats, softmax) is computed on device from
Gx/stats; the 16384-wide tensor is touched exactly twice (Gram+stats pass,
final affine pass).
"""

import numpy as np

import concourse.bacc as bacc
import concourse.tile as tile
from concourse import mybir
from concourse.bass_utils import run_bass_kernel_spmd

C = 128          # channels
N = 16384        # spatial (H*W)
GROUPS = 8
GS = C // GROUPS  # 16 channels per group
HEADS = 8
HD = C // HEADS   # 16
EPS = 1e-5
SCALE = HD ** -0.5  # 0.25

F32 = mybir.dt.float32
F32R = mybir.dt.float32r
BF16 = mybir.dt.bfloat16

# chunk sizes
DMA_CHUNK = 2048          # x DMA-in granularity
BN_CHUNK = 512            # bn_stats hardware max
TR_GROUP = 512            # 4x 128-col transposes per PSUM bank
OUT_CHUNK = 512           # phase-3 matmul free dim (one PSUM bank)
OUT_DMA = 2048            # phase-3 DMA-out granularity

# Native-f32r mode: x and ident are declared float32r end-to-end (same fp32
# bits from the host; DMA is the "rounding" producer walrus requires), making
# the transposes 1.5 cyc/row and the phase-3 matmul 1 cyc/row instead of
# fp32's 2/4. The residual path stays exact: x is re-added in fp32 during
# evacuation, so only the conv term sees f32r rounding.
F32R_NATIVE = True


def build_nc():
    nc = bacc.Bacc(None, target_bir_lowering=False, debug=True)

    XDT = F32R if F32R_NATIVE else F32
    x_dram = nc.dram_tensor("x_img", (C, N), XDT, kind="ExternalInput")
    y_dram = nc.dram_tensor("y_img", (C, N), F32, kind="ExternalOutput")
    # all fp32 constants packed into one blob (ident rides in it via bitcast)
    consts_d = nc.dram_tensor("consts", (C, NCON), F32, kind="ExternalInput")

    with tile.TileContext(nc) as tc:
        with tc.tile_pool(name="persist", bufs=1) as persist:
            # ---- constants / weights: one blob tile, sliced ----
            consts = persist.tile([C, NCON], F32, tag="consts")  # noqa: F841
            ident = persist.tile([C, C], XDT, tag="ident")
            w_inT = consts[:, 0 * C:1 * C]
            w_inF = consts[:, 1 * C:2 * C]
            w_outT = consts[:, 2 * C:3 * C]
            amask = consts[:, 3 * C:4 * C]
            smask = consts[:, 4 * C:5 * C]
            gnw = consts[:, 5 * C:5 * C + 1]
            gnb = consts[:, 5 * C + 1:5 * C + 2]
            eps_col = consts[:, 5 * C + 2:5 * C + 3]
            RB = 5 * C + 4
            binr = consts[0:1, RB:RB + C]
            boutr = consts[0:1, RB + C:RB + 2 * C]
            ones11 = consts[0:1, RB + 2 * C:RB + 2 * C + 1]

            # ---- persistent state ----
            n_dma = N // DMA_CHUNK
            n_bn = N // BN_CHUNK
            x_chunks = []
            for d in range(n_dma):
                x_chunks.append(persist.tile([C, DMA_CHUNK], XDT, tag=f"x{d}", name=f"x_sb{d}"))
            stats = persist.tile([C, n_bn, 6], F32, tag="stats")
            gxp_cm = tc.tile_pool(name="gxp", bufs=1, space="PSUM")
            gxp = gxp_cm.__enter__()
            gx_psum = gxp.tile([C, C], F32, tag="gx")

            # =========== PHASE 1: DMA in + stats + transpose + Gram ===========
            n_tg = N // TR_GROUP          # transpose groups
            TPG = TR_GROUP // C           # transposes per group
            tg_per_dma = DMA_CHUNK // TR_GROUP
            bn_per_dma = DMA_CHUNK // BN_CHUNK

            def dma_x_chunk(d):
                base = d * DMA_CHUNK
                if d in (0, n_dma - 1):
                    # sub-DMAs so edge compute starts (finishes) earlier
                    for off, w in ((0, 512), (512, 512), (1024, 1024)) if d == 0                             else ((0, 1024), (1024, 512), (1536, 512)):
                        nc.sync.dma_start(out=x_chunks[d][:, off:off + w],
                                          in_=x_dram[:, base + off:base + off + w])
                else:
                    nc.sync.dma_start(out=x_chunks[d], in_=x_dram[:, base:base + DMA_CHUNK])

            id_src = consts_d[:, IDC:IDC + C]
            if F32R_NATIVE:
                id_src = id_src.bitcast(F32R)
            nc.sync.dma_start(out=ident, in_=id_src)
            dma_x_chunk(0)
            with (
                tc.tile_pool(name="trp", bufs=5, space="PSUM") as trp,
                tc.tile_pool(name="xtp", bufs=8) as xtp,
            ):
                for d in range(n_dma):
                    xc = x_chunks[d]
                    if d > 0:
                        dma_x_chunk(d)
                    if d == n_dma - 1:
                        # consts only needed by phase 2; keep out of the x stream
                        nc.sync.dma_start(out=consts, in_=consts_d[:])
                    xcf = xc.bitcast(F32) if F32R_NATIVE else xc
                    for k in range(bn_per_dma):
                        nc.vector.bn_stats(
                            out=stats[:, d * bn_per_dma + k, :],
                            in_=xcf[:, k * BN_CHUNK:(k + 1) * BN_CHUNK],
                        )
                    for g in range(tg_per_dma):
                        tg = d * tg_per_dma + g  # global transpose-group idx
                        ps_tr = trp.tile([C, TPG, C], F32, tag="ps_tr")
                        for t in range(TPG):
                            off = g * TR_GROUP + t * C
                            dst = ps_tr[:, t, :]
                            if F32R_NATIVE:
                                dst = dst.bitcast(F32R)
                            nc.tensor.transpose(dst, xc[:, off:off + C], ident)
                        xt = xtp.tile([C, TPG, C], BF16, tag="xt")
                        if tg % 5 == 4:
                            # spread some PSUM evacuations onto DVE's slack so
                            # ACT (the phase-1 straggler) finishes sooner
                            nc.vector.tensor_copy(out=xt, in_=ps_tr)
                        else:
                            nc.scalar.copy(out=xt, in_=ps_tr)
                        for t in range(TPG):
                            gi = tg * TPG + t
                            nc.tensor.matmul(
                                gx_psum, xt[:, t, :], xt[:, t, :],
                                start=(gi == 0), stop=(gi == n_tg * TPG - 1),
                            )

            # =========== PHASE 2: small algebra ===========
            sm = persist  # small persistent tiles
            with tc.tile_pool(name="ps2", bufs=3, space="PSUM") as ps2:
                # channel stats -> group stats (replicated per channel)
                mv = sm.tile([C, 2], F32, tag="mv")
                nc.vector.bn_aggr(out=mv, in_=stats)
                mq = sm.tile([C, 2], F32, tag="mq")
                nc.vector.tensor_copy(out=mq[:, 0:1], in_=mv[:, 0:1])
                # E[x^2]_c = var_c + mean_c^2
                nc.vector.tensor_mul(out=mq[:, 1:2], in0=mv[:, 0:1], in1=mv[:, 0:1])
                nc.vector.tensor_add(out=mq[:, 1:2], in0=mq[:, 1:2], in1=mv[:, 1:2])
                mg_ps = ps2.tile([C, 2], F32, tag="ps2")
                nc.tensor.matmul(mg_ps, amask, mq, start=True, stop=True)
                mg = sm.tile([C, 2], F32, tag="mg")
                nc.vector.tensor_copy(out=mg, in_=mg_ps)

                # a = gn_w * rsqrt(var_g + eps);  b2 = gn_b - mean_g * a
                # varg_neg = mean_g^2 - E[x^2]_g  (= -var); Sqrt(scale=-1) flips it
                varg = sm.tile([C, 1], F32, tag="varg")
                nc.vector.scalar_tensor_tensor(
                    out=varg, in0=mg[:, 0:1], scalar=mg[:, 0:1], in1=mg[:, 1:2],
                    op0=mybir.AluOpType.mult, op1=mybir.AluOpType.subtract)
                a_col = sm.tile([C, 1], F32, tag="a_col")
                nc.scalar.activation(out=a_col, in_=varg,
                                     func=mybir.ActivationFunctionType.Sqrt,
                                     bias=eps_col, scale=-1.0)
                nc.vector.reciprocal(out=a_col, in_=a_col)
                nc.vector.tensor_mul(out=a_col, in0=a_col, in1=gnw)
                b2_col = sm.tile([C, 1], F32, tag="b2_col")
                nc.vector.tensor_mul(out=b2_col, in0=mg[:, 0:1], in1=a_col)
                nc.vector.tensor_sub(out=b2_col, in0=gnb, in1=b2_col)

                # Mt[c, m] = w_inT[c, m] * a[c]
                mt = sm.tile([C, C], F32, tag="mt")
                nc.vector.tensor_scalar_mul(out=mt, in0=w_inT, scalar1=a_col)

                # Sx column (channel sums) = mean_c * N
                sx_col = sm.tile([C, 1], F32, tag="sx_col")
                nc.vector.tensor_scalar_mul(out=sx_col, in0=mv[:, 0:1], scalar1=float(N))

                # Gx -> SBUF (DVE, so ACT can prefetch act-func tables)
                gx_sb = sm.tile([C, C], F32, tag="gx_sb")
                nc.vector.tensor_copy(out=gx_sb, in_=gx_psum)

                # T1 = Gx @ Mt   (Gx symmetric)
                t1_ps = ps2.tile([C, C], F32, tag="ps2")
                nc.tensor.matmul(t1_ps, gx_sb, mt, start=True, stop=True)
                t1 = sm.tile([C, C], F32, tag="t1")
                nc.vector.tensor_copy(out=t1, in_=t1_ps)

                # b' row and column:  b' = W_in @ b2 + b_in
                bp_ps = ps2.tile([1, C], F32, tag="ps2")
                nc.tensor.matmul(bp_ps, b2_col, w_inT, start=True, stop=False)
                nc.tensor.matmul(bp_ps, ones11, binr, start=False, stop=True)
                bp_row = sm.tile([1, C], F32, tag="bp_row")
                nc.vector.tensor_copy(out=bp_row, in_=bp_ps)
                bpc_ps = ps2.tile([C, 1], F32, tag="ps2")
                nc.tensor.matmul(bpc_ps, w_inT, b2_col, start=True, stop=False)
                nc.tensor.matmul(bpc_ps, binr, ones11, start=False, stop=True)
                bp_col = sm.tile([C, 1], F32, tag="bp_col")
                nc.vector.tensor_copy(out=bp_col, in_=bpc_ps)

                # v row = (M @ Sx)^T
                v_ps = ps2.tile([1, C], F32, tag="ps2")
                nc.tensor.matmul(v_ps, sx_col, mt, start=True, stop=True)
                v_row = sm.tile([1, C], F32, tag="v_row")
                nc.vector.tensor_copy(out=v_row, in_=v_ps)
                bpn_row = sm.tile([1, C], F32, tag="bpn_row")
                nc.vector.tensor_scalar_mul(out=bpn_row, in0=bp_row, scalar1=float(N))

                # gram = Mt^T Gx Mt + v b'^T + b' v^T + N b' b'^T
                gram_ps = ps2.tile([C, C], F32, tag="ps2")
                nc.tensor.matmul(gram_ps, t1, mt, start=True, stop=False)
                nc.tensor.matmul(gram_ps, v_row, bp_row, start=False, stop=False)
                nc.tensor.matmul(gram_ps, bp_row, v_row, start=False, stop=False)
                nc.tensor.matmul(gram_ps, bpn_row, bp_row, start=False, stop=True)
                # masked full-row softmax: scores*SCALE + (-30000 off-block)
                scores = sm.tile([C, C], F32, tag="scores")
                nc.vector.scalar_tensor_tensor(
                    out=scores, in0=gram_ps, scalar=SCALE, in1=smask,
                    op0=mybir.AluOpType.mult, op1=mybir.AluOpType.add)
                rmax = sm.tile([C, 1], F32, tag="rmax")
                nc.vector.reduce_max(out=rmax, in_=scores, axis=mybir.AxisListType.X,
                                     negate=True)
                wsm = sm.tile([C, C], F32, tag="wsm")
                ssum = sm.tile([C, 1], F32, tag="ssum")
                nc.scalar.activation(out=wsm, in_=scores,
                                     func=mybir.ActivationFunctionType.Exp,
                                     bias=rmax, scale=1.0, accum_out=ssum)
                nc.vector.reciprocal(out=ssum, in_=ssum)
                nc.vector.tensor_scalar_mul(out=wsm, in0=wsm, scalar1=ssum)

                # P1 = W_sm^T @ w_out^T  (= W_comb^T)
                p1_ps = ps2.tile([C, C], F32, tag="ps2")
                nc.tensor.matmul(p1_ps, wsm, w_outT, start=True, stop=True)
                p1 = sm.tile([C, C], F32, tag="p1")
                nc.vector.tensor_copy(out=p1, in_=p1_ps)

                # W_finT = Mt @ P1 = diag(a) . (w_inT @ P1); the host-side
                # untransposed w_in is exactly the lhsT for w_inT @ P1, and the
                # diag(a) row scaling rides on the PSUM evacuation. (Identity is
                # NOT folded; residual is added in fp32 during phase-3 evac.)
                wt_ps = ps2.tile([C, C], F32, tag="ps2")
                nc.tensor.matmul(wt_ps, w_inF, p1, start=True, stop=True)
                wtot = sm.tile([C, C], XDT, tag="wtot")
                nc.vector.tensor_scalar_mul(out=wtot, in0=wt_ps, scalar1=a_col)

                # b_fin column = W_comb @ b' + b_out
                bf_ps = ps2.tile([C, 1], F32, tag="ps2")
                nc.tensor.matmul(bf_ps, p1, bp_col, start=True, stop=False)
                nc.tensor.matmul(bf_ps, boutr, ones11, start=False, stop=True)
                bfin = sm.tile([C, 1], F32, tag="bfin")
                nc.vector.tensor_copy(out=bfin, in_=bf_ps)

            gxp_cm.__exit__(None, None, None)

            # ===== PHASE 3: out = x + W_finT^T x + b_fin (streamed) =====
            oc_per_dma = OUT_DMA // OUT_CHUNK
            with (
                tc.tile_pool(name="po", bufs=2, space="PSUM") as po,
                tc.tile_pool(name="ob", bufs=3) as obp,
            ):
                for d in range(N // OUT_DMA):
                    ot = obp.tile([C, OUT_DMA], F32, tag="ot")
                    ops = po.tile([C, OUT_DMA], F32, tag="ops")
                    xs = x_chunks[(d * OUT_DMA) // DMA_CHUNK]
                    lo = (d * OUT_DMA) % DMA_CHUNK
                    for k in range(oc_per_dma):
                        nc.tensor.matmul(
                            ops[:, k * OUT_CHUNK:(k + 1) * OUT_CHUNK], wtot,
                            xs[:, lo + k * OUT_CHUNK:lo + (k + 1) * OUT_CHUNK],
                            start=True, stop=True)
                    xres = xs[:, lo:lo + OUT_DMA]
                    if F32R_NATIVE:
                        xres = xres.bitcast(F32)
                    # out = (W_finT^T x + b_fin) + x   (residual exact fp32)
                    if d <= 7:
                        # narrow evacs + DMAs so the out-stream starts sooner
                        for off, w in (((0, 512), (512, 512), (1024, 1024)) if d == 0
                                       else ((0, 1024), (1024, 1024))):
                            sl = slice(off, off + w)
                            nc.vector.scalar_tensor_tensor(
                                out=ot[:, sl], in0=ops[:, sl], scalar=bfin,
                                in1=xres[:, sl],
                                op0=mybir.AluOpType.add, op1=mybir.AluOpType.add)
                            nc.sync.dma_start(
                                out=y_dram[:, d * OUT_DMA + off:d * OUT_DMA + off + w],
                                in_=ot[:, sl])
                    else:
                        nc.vector.scalar_tensor_tensor(
                            out=ot, in0=ops, scalar=bfin, in1=xres,
                            op0=mybir.AluOpType.add, op1=mybir.AluOpType.add)
                        nc.sync.dma_start(out=y_dram[:, d * OUT_DMA:(d + 1) * OUT_DMA], in_=ot)

    nc.compile()
    return nc


IDC = 5 * C + 4 + 2 * C + 1
NCON = IDC + C


def host_weights(gn_w, gn_b, w_in, b_in, w_out, b_out):
    amask = np.zeros((C, C), dtype=np.float32)
    for g in range(GROUPS):
        amask[g * GS:(g + 1) * GS, g * GS:(g + 1) * GS] = 1.0 / GS
    smask = np.full((C, C), -30000.0, dtype=np.float32)
    for h in range(HEADS):
        smask[h * HD:(h + 1) * HD, h * HD:(h + 1) * HD] = 0.0
    blob = np.zeros((C, NCON), dtype=np.float32)
    blob[:, 0 * C:1 * C] = w_in.T
    blob[:, 1 * C:2 * C] = w_in
    blob[:, 2 * C:3 * C] = w_out.T
    blob[:, 3 * C:4 * C] = amask
    blob[:, 4 * C:5 * C] = smask
    blob[:, 5 * C] = gn_w
    blob[:, 5 * C + 1] = gn_b
    blob[:, 5 * C + 2] = EPS
    RB = 5 * C + 4
    blob[0, RB:RB + C] = b_in
    blob[0, RB + C:RB + 2 * C] = b_out
    blob[0, RB + 2 * C] = 1.0
    blob[:, IDC:IDC + C] = np.eye(C, dtype=np.float32)
    return {"consts": blob}


_NC_CACHE = None


def kernel(x, gn_w, gn_b, w_in, b_in, w_out, b_out):
    global _NC_CACHE
    x = np.asarray(x, dtype=np.float32)
    B = x.shape[0]
    assert x.shape == (B, C, 128, 128) and B == 8
    if _NC_CACHE is None:
        _NC_CACHE = build_nc()
    nc = _NC_CACHE
    w = host_weights(np.asarray(gn_w), np.asarray(gn_b), np.asarray(w_in),
                     np.asarray(b_in), np.asarray(w_out), np.asarray(b_out))
    in_maps = []
    for b in range(B):
        m = dict(w)
        m["x_img"] = np.ascontiguousarray(x[b].reshape(C, N))
        in_maps.append(m)
    res = run_bass_kernel_spmd(nc, in_maps, core_ids=list(range(B)))
    out = np.stack([res.results[b]["y_img"].reshape(C, 128, 128) for b in range(B)])
    return out.astype(np.float32)

